# revision 1
# baseline (speedup 1.0000x reference)
"""MoE router (GroupBRouter) Trainium2 Bass kernel.

Computes gates = top2_mask(hard_cap(floor_lerp(softmax(tokens @ W_g.T + b_g)), t))
for tokens (16, 4096, 1024) f32, sharded 2 batches per core across 8 cores.

Layout strategy:
  - Host transposes each core's token shard to [D=1024, T=8192] so the PE
    can consume [128-row D-chunk, 128-token] stationary blocks straight from
    contiguous DMA loads.
  - W_g is pre-arranged host-side to [128, 8, 64]  (w[p, c, e] = W_g[e, c*128+p]).
  - Matmul: lhsT = token block [K=128 d, M=128 tokens], rhs = W chunk
    [K=128 d, N=64 experts] -> PSUM [128 tokens, 64 experts], accumulated
    over the 8 D-chunks. 8 token-groups share one PSUM bank tile [128, 8, 64].
  - Softmax/cap/top2 run on [128, 8, 64] tiles with grouped (axis=X) reduces.

Math notes exploited:
  - cap = 0.5 + 1.1*t/1000 >= 0.5 and probs sum to 1 with a strictly positive
    floor, so at most ONE expert can exceed the cap -> no exact ties from
    capping; a threshold-based top-2 (mask = capped2 >= second_max) matches
    jax.lax.top_k up to measure-zero float coincidences.
  - headroom is kept negated (nhr = min(p - cap, 0) = -relu(cap - p)) because
    tensor_scalar has no reverse-subtract; signs cancel in the redistribution
    term: hr*ratio = nhr * (-ratio).
  - all capped2 values are >= floor > 0, so masking the argmax by multiplying
    with (capped2 < max) zeroes it, and a plain max finds the runner-up.
"""

import numpy as np

_B, _N, _D, _E = 16, 4096, 1024, 64
_NCORES = 8
_B_LOC = _B // _NCORES          # 2 batches per core
_T_CORE = _B_LOC * _N           # 8192 tokens per core
_NCHUNK = _D // 128             # 8 D-chunks
_ST_TOK = 1024                  # tokens per supertile
_NST = _T_CORE // _ST_TOK       # 8 supertiles per core
_NGRP = _ST_TOK // 128          # 8 token-groups of 128 per supertile

_FLOOR_C = np.float32(0.15 / 64.0)   # alpha/e
_FLOOR_M = np.float32(1.0 - 0.15)    # 1 - alpha

_cached = {}


def _patch_single_swdge_lane():
    # Route every SWDGE DMA through one completion-semaphore lane. Same-lane
    # DMAs are FIFO-ordered (one proc in Tile's vector clock), so the
    # redundant DMA-to-DMA WAW waits disappear and each DMA carries at most
    # one sync wait — the TPB instruction encoding has a single wait slot,
    # and this toolchain's walrus rejects instructions needing more.
    from concourse import tile_sem_assignment as tsa
    if getattr(tsa.TileClockTick, "_single_swdge", False):
        return
    orig = tsa.TileClockTick.__init__

    def patched(self, *a, **k):
        orig(self, *a, **k)
        self.swdge_sem_count = 1

    tsa.TileClockTick.__init__ = patched
    tsa.TileClockTick._single_swdge = True


def _build_program():
    import concourse.bass as bass
    import concourse.tile as tile
    from concourse import mybir

    _patch_single_swdge_lane()

    f32 = mybir.dt.float32
    Alu = mybir.AluOpType
    Act = mybir.ActivationFunctionType
    X = mybir.AxisListType.X

    nc = bass.Bass("TRN2", enable_partition_id=False)

    tokT_h = nc.dram_tensor("tokT", (_D, _T_CORE), f32, kind="ExternalInput")
    w_h = nc.dram_tensor("w", (128, _NCHUNK, _E), f32, kind="ExternalInput")
    cap_h = nc.dram_tensor("cap", (128, _B_LOC), f32, kind="ExternalInput")
    bias_h = nc.dram_tensor("bias", (128, _E), f32, kind="ExternalInput")
    out_h = nc.dram_tensor("gates", (_T_CORE, _E), f32, kind="ExternalOutput")

    with tile.TileContext(nc) as tc:
        with tc.tile_pool(name="singles", bufs=1) as singles, \
             tc.tile_pool(name="tok", bufs=2) as tokp, \
             tc.tile_pool(name="big", bufs=2) as big, \
             tc.tile_pool(name="stats", bufs=2) as stats, \
             tc.tile_pool(name="psum", bufs=8, space="PSUM") as psump:

            w_t = singles.tile([128, _NCHUNK, _E], f32)
            nc.sync.dma_start(out=w_t, in_=w_h[:, :, :])
            cap_t = singles.tile([128, _B_LOC], f32)
            nc.sync.dma_start(out=cap_t, in_=cap_h[:, :])
            bias_t = singles.tile([128, _E], f32)
            nc.sync.dma_start(out=bias_t, in_=bias_h[:, :])

            # Consume the cap/bias DMA semaphores with cheap DVE copies so no
            # later DVE op needs two sync waits (one per DMA lane) at once.
            scratch = singles.tile([128, 2], f32)
            nc.vector.tensor_copy(scratch[:, 0:1], cap_t[:, 0:1])
            nc.vector.tensor_copy(scratch[:, 1:2], bias_t[:, 0:1])

            for st in range(_NST):
                tok = tokp.tile([128, _NCHUNK, _ST_TOK], f32)
                src = tokT_h[:, st * _ST_TOK:(st + 1) * _ST_TOK].rearrange(
                    "(c p) t -> p c t", p=128)
                nc.sync.dma_start(out=tok, in_=src)

                ps = psump.tile([128, _NGRP, _E], f32)
                if st == 0:
                    # Dummy matmul reading only w_t: absorbs the w-DMA wait on
                    # the PE so real matmuls each carry a single sync wait
                    # (their token-DMA semaphore). Matmult supports only one.
                    nc.tensor.matmul(
                        ps[0:_E, 0, 0:1], w_t[:, 0, :], w_t[:, 0, 0:1],
                        start=True, stop=True, skip_group_check=True)
                for tt in range(_NGRP):
                    for c in range(_NCHUNK):
                        nc.tensor.matmul(
                            ps[:, tt, :],
                            tok[:, c, tt * 128:(tt + 1) * 128],
                            w_t[:, c, :],
                            start=(c == 0),
                            stop=(c == _NCHUNK - 1),
                        )

                capb = cap_t[:, (st // (_N // _ST_TOK)):(st // (_N // _ST_TOK)) + 1]
                shp = [128, _NGRP, _E]

                def bc(s):  # [128, G] -> [128, G, E] stride-0 broadcast
                    return s[:, :, None].broadcast_to(shp)

                lg = big.tile(shp, f32)    # logits + bias
                nc.vector.tensor_tensor(
                    lg, ps, bias_t[:, None, :].broadcast_to(shp), Alu.add)
                mx = stats.tile([128, _NGRP], f32)
                nc.vector.tensor_reduce(mx, lg, X, Alu.max)
                xm = big.tile(shp, f32)
                nc.vector.tensor_tensor(xm, lg, bc(mx), Alu.subtract)
                ex = big.tile(shp, f32)
                nc.scalar.activation(ex, xm, Act.Exp)
                s_ = stats.tile([128, _NGRP], f32)
                nc.vector.tensor_reduce(s_, ex, X, Alu.add)
                r_ = stats.tile([128, _NGRP], f32)
                nc.vector.reciprocal(r_, s_)
                q = big.tile(shp, f32)
                nc.vector.tensor_tensor(q, ex, bc(r_), Alu.mult)
                p_ = big.tile(shp, f32)    # floored probs
                nc.vector.tensor_scalar(
                    p_, q, float(_FLOOR_M), float(_FLOOR_C), Alu.mult, Alu.add)

                exs = big.tile(shp, f32)   # excess = relu(p - cap)
                nc.vector.tensor_scalar(exs, p_, capb, 0.0, Alu.subtract, Alu.max)
                nhr = big.tile(shp, f32)   # -headroom = min(p - cap, 0)
                nc.vector.tensor_scalar(nhr, p_, capb, 0.0, Alu.subtract, Alu.min)
                cpd = big.tile(shp, f32)   # capped = min(p, cap)
                nc.vector.tensor_scalar(cpd, p_, capb, None, Alu.min)

                exsum = stats.tile([128, _NGRP], f32)
                nc.vector.tensor_reduce(exsum, exs, X, Alu.add)
                nhrsum = stats.tile([128, _NGRP], f32)  # -headroom_sum
                nc.vector.tensor_reduce(nhrsum, nhr, X, Alu.add)
                nhrc = stats.tile([128, _NGRP], f32)    # -max(hr_sum, 1e-8)
                nc.vector.tensor_scalar(nhrc, nhrsum, -1e-8, None, Alu.min)
                nhri = stats.tile([128, _NGRP], f32)    # -1/clip(hr_sum)
                nc.vector.reciprocal(nhri, nhrc)
                nrat = stats.tile([128, _NGRP], f32)    # -excess_sum/clip(hr_sum)
                nc.vector.tensor_tensor(nrat, exsum, nhri, Alu.mult)

                hrr = big.tile(shp, f32)   # headroom * ratio = nhr * nrat
                nc.vector.tensor_tensor(hrr, nhr, bc(nrat), Alu.mult)
                c2 = big.tile(shp, f32)    # capped2 (final prob vector)
                nc.vector.tensor_tensor(c2, cpd, hrr, Alu.add)

                m1 = stats.tile([128, _NGRP], f32)
                nc.vector.tensor_reduce(m1, c2, X, Alu.max)
                i1 = big.tile(shp, f32)    # 0 at argmax, 1 elsewhere
                nc.vector.tensor_tensor(i1, c2, bc(m1), Alu.is_lt)
                c3 = big.tile(shp, f32)    # c2 with argmax zeroed (all vals > 0)
                nc.vector.tensor_tensor(c3, c2, i1, Alu.mult)
                m2 = stats.tile([128, _NGRP], f32)
                nc.vector.tensor_reduce(m2, c3, X, Alu.max)
                msk = big.tile(shp, f32)   # top-2 mask
                nc.vector.tensor_tensor(msk, c2, bc(m2), Alu.is_ge)
                g = big.tile(shp, f32)
                nc.vector.tensor_tensor(g, c2, msk, Alu.mult)

                dst = out_h[st * _ST_TOK:(st + 1) * _ST_TOK, :].rearrange(
                    "(tt p) e -> p tt e", p=128)
                nc.gpsimd.dma_start(out=dst, in_=g)

    _strip_redundant_waits(nc, mybir)
    return nc


def _strip_redundant_waits(nc, mybir):
    """Reduce every non-Drain instruction to <=1 sync wait.

    The TPB instruction encoding has one wait slot; this walrus rejects more.
    Two provably-redundant classes are dropped:
      A. own-engine waits: engines are strict-FIFO, so an instruction never
         needs a semaphore wait on its own engine's stream (Tile's Rust wait
         pass emits these conservatively).
      B. DMA-lane waits on a DMA that also waits on PE/DVE: the engine wait
         transitively implies the old same-slot DMA completed, because those
         engine instructions gated on that DMA's semaphore and read the data
         (Tile doesn't track cross-proc transitivity). All SWDGE DMAs share
         one lane here (see _patch_single_swdge_lane), and same-slot rewrites
         target the same partitions -> same SDMA engines -> FIFO per engine.
    """
    eng_sem = {
        "EngineType.Activation": "Activation_",
        "EngineType.DVE": "DVE_",
        "EngineType.PE": "PE_",
        "EngineType.SP": "SP_",
    }
    for name, ins in nc.inst_map.items():
        si = ins.sync_info
        if not si or not si.on_wait or len(si.on_wait) < 2:
            continue
        if type(ins).__name__ == "InstDrain":
            # Tail barrier: the last out-DMA's lane semaphore transitively
            # implies every engine sem (all compute feeds the output DMAs,
            # which are the final SWDGE lane ticks).
            waits = [w for w in si.on_wait if w.ant_name.startswith("DMASW0")]
            assert waits, name
            ins.sync_info = mybir.SyncInfo(
                on_wait=waits[-1:], on_update=list(si.on_update))
            continue
        waits = list(si.on_wait)
        own = eng_sem.get(str(ins.engine))
        if own is not None:
            waits = [w for w in waits if not w.ant_name.startswith(own)]
        if type(ins).__name__ == "InstDMACopy" and len(waits) >= 2:
            engw = [w for w in waits if w.ant_name.startswith(("PE_", "DVE_"))]
            if engw:
                waits = engw[:1]
        assert len(waits) <= 1, (name, [(w.ant_name, w.wait_value) for w in waits])
        ins.sync_info = mybir.SyncInfo(on_wait=waits, on_update=list(si.on_update))


def _get_program():
    if "nc" not in _cached:
        _cached["nc"] = _build_program()
    return _cached["nc"]


def kernel(tokens_B, t, W_g, b_g):
    from concourse import bass_utils

    tokens_B = np.ascontiguousarray(np.asarray(tokens_B, dtype=np.float32))
    t = np.asarray(t, dtype=np.int32)
    W_g = np.asarray(W_g, dtype=np.float32)
    b_g = np.asarray(b_g, dtype=np.float32)

    # W_g (E, D) -> [128, NCHUNK, E]: w[p, c, e] = W_g[e, c*128+p]
    w_prep = np.ascontiguousarray(
        W_g.T.reshape(_NCHUNK, 128, _E).transpose(1, 0, 2))
    bias_prep = np.ascontiguousarray(
        np.broadcast_to(b_g[None, :], (128, _E)))

    # cap in f32 with the same op order as the reference
    t_norm = t.astype(np.float32) / np.float32(1000.0)
    cap_all = np.float32(0.5) + np.float32(1.1) * t_norm   # (B,)

    in_maps = []
    for j in range(_NCORES):
        shard = tokens_B[j * _B_LOC:(j + 1) * _B_LOC]      # (2, 4096, 1024)
        tokT = np.ascontiguousarray(
            shard.transpose(2, 0, 1).reshape(_D, _T_CORE))
        cap_prep = np.ascontiguousarray(np.broadcast_to(
            cap_all[j * _B_LOC:(j + 1) * _B_LOC][None, :], (128, _B_LOC)))
        in_maps.append({
            "tokT": tokT,
            "w": w_prep,
            "cap": cap_prep,
            "bias": bias_prep,
        })

    nc = _get_program()
    res = bass_utils.run_bass_kernel_spmd(nc, in_maps, list(range(_NCORES)))

    out = np.empty((_B, _N, _E), dtype=np.float32)
    for j in range(_NCORES):
        out[j * _B_LOC:(j + 1) * _B_LOC] = \
            res.results[j]["gates"].reshape(_B_LOC, _N, _E)
    return out

